# revision 1
# baseline (speedup 1.0000x reference)
"""Cross-attention kernel for 8 Trainium2 NeuronCores.

Sharding: 8 cores = 4 batches x 2 head-groups (6 heads each).
Per core (b, hg), with all activations pre-transposed on host:
  qT = (Wq_hg*scale).T' @ xqT   [384, 2048]   (weights column-split)
  kT = Wk_hg' @ xkT             [384, 2048]
  v  = xvT' @ Wv_hg.T           [2048, 384]  (+ ones column per head)
  per head h: lt = k_h qT_h     [2048k, 2048q] (logits transposed)
              p  = exp(lt)      (no max-subtraction: logits are O(1))
              [x; d] = [v_h|1].T @ p   -> x rows 0..63, denominators row 64
              xn = x * (1/d)    (partition-broadcast of 1/d)
  outT_partial = Wo_hg.T' @ xn  [768, 2048]
Host: out[b] = (partial[2b] + partial[2b+1]).T + bo.

All matmuls bf16 inputs with fp32 PSUM accumulation.
"""

import sys

import numpy as np

for _p in ("/opt/trn_rl_repo",):
    if _p not in sys.path:
        sys.path.insert(0, _p)

B, NQ, NK, C = 4, 2048, 2048, 768
H, DH = 12, 64
HPC, HB = 6, 384  # heads per core, head-block width
P = 128
KT = C // P  # 6 contraction k-tiles for projections
QCH = 512  # query-chunk width
NCH = NQ // QCH  # 4 query chunks
NKT = NK // P  # 16 key tiles
SCALE = DH**-0.5  # folded into Wq on host (exactly 0.125)
VW = DH + 1  # v block width per head incl. ones column

_prog = None


def _build():
    from contextlib import ExitStack

    import concourse.bass as bass
    import concourse.tile as tile
    from concourse import mybir
    from concourse.bacc import Bacc

    f32 = mybir.dt.float32
    bf16 = mybir.dt.bfloat16
    EXP = mybir.ActivationFunctionType.Exp

    nc = Bacc()
    xq_d = nc.declare_dram_parameter("xq", [C, NQ], bf16, isOutput=False)
    xk_d = nc.declare_dram_parameter("xk", [C, NK], bf16, isOutput=False)
    xv_d = nc.declare_dram_parameter("xv", [C, NK], bf16, isOutput=False)
    wq_d = nc.declare_dram_parameter("wq", [C, HB], bf16, isOutput=False)
    wk_d = nc.declare_dram_parameter("wk", [C, HB], bf16, isOutput=False)
    wv_d = nc.declare_dram_parameter("wv", [C, HB], bf16, isOutput=False)
    wo_d = nc.declare_dram_parameter("wo", [HB, C], bf16, isOutput=False)
    out_d = nc.declare_dram_parameter("out", [C, NQ], f32, isOutput=True)

    with tile.TileContext(nc) as tc, ExitStack() as ctx:
        const = ctx.enter_context(tc.tile_pool(name="const", bufs=1))
        xin = ctx.enter_context(tc.tile_pool(name="xin", bufs=KT))
        qk = ctx.enter_context(tc.tile_pool(name="qk", bufs=3))
        pp = ctx.enter_context(tc.tile_pool(name="pp", bufs=16))
        xnp = ctx.enter_context(tc.tile_pool(name="xnp", bufs=3))
        small = ctx.enter_context(tc.tile_pool(name="small", bufs=2))
        ost = ctx.enter_context(tc.tile_pool(name="ost", bufs=3))
        drp = ctx.enter_context(tc.tile_pool(name="drp", bufs=2, space="DRAM"))
        proj_ps = ctx.enter_context(tc.tile_pool(name="proj_ps", bufs=2, space="PSUM"))
        lt_ps = ctx.enter_context(tc.tile_pool(name="lt_ps", bufs=2, space="PSUM"))
        x_ps = ctx.enter_context(tc.tile_pool(name="x_ps", bufs=2, space="PSUM"))

        # ---- weights + inputs to SBUF
        wq_s = const.tile([P, KT, HB], bf16, tag="wq")
        wk_s = const.tile([P, KT, HB], bf16, tag="wk")
        wv_s = const.tile([P, KT, HB], bf16, tag="wv")
        wo_s = const.tile([P, HB // P, C], bf16, tag="wo")
        nc.sync.dma_start(out=wq_s, in_=wq_d.rearrange("(k p) m -> p k m", p=P))
        nc.sync.dma_start(out=wk_s, in_=wk_d.rearrange("(k p) m -> p k m", p=P))
        nc.sync.dma_start(out=wv_s, in_=wv_d.rearrange("(k p) m -> p k m", p=P))
        nc.sync.dma_start(out=wo_s, in_=wo_d.rearrange("(k p) m -> p k m", p=P))

        xq_t, xk_t, xv_t = [], [], []
        for k in range(KT):
            for name, dram, lst in (
                ("xq", xq_d, xq_t),
                ("xk", xk_d, xk_t),
                ("xv", xv_d, xv_t),
            ):
                t = xin.tile([P, NQ], bf16, tag=name, name=f"{name}_{k}")
                nc.sync.dma_start(
                    out=t, in_=dram.rearrange("(k p) m -> p k m", p=P)[:, k, :]
                )
                lst.append(t)

        # v with a ones column per head: [128, kt, head, 65]
        v_s = const.tile([P, NKT, HPC, VW], bf16, tag="v")
        nc.vector.memset(v_s[:, :, :, DH : DH + 1], 1.0)

        qT_t = [qk.tile([P, NQ], bf16, tag="qT", name=f"qT{i}") for i in range(3)]
        kT_t = [qk.tile([P, NK], bf16, tag="kT", name=f"kT{i}") for i in range(3)]

        def proj_qk(w_s, src, dst, mt):
            for j4 in range(NCH):
                ps = proj_ps.tile([P, QCH], f32, tag="proj")
                for k in range(KT):
                    nc.tensor.matmul(
                        ps,
                        w_s[:, k, mt * P : (mt + 1) * P],
                        src[k][:, j4 * QCH : (j4 + 1) * QCH],
                        start=(k == 0),
                        stop=(k == KT - 1),
                    )
                nc.vector.tensor_copy(dst[:, j4 * QCH : (j4 + 1) * QCH], ps)

        # projections for head-pair 0 first so ACT can start early
        proj_qk(wq_s, xq_t, qT_t[0], 0)
        proj_qk(wk_s, xk_t, kT_t[0], 0)
        for mt in (1, 2):
            proj_qk(wq_s, xq_t, qT_t[mt], mt)
            proj_qk(wk_s, xk_t, kT_t[mt], mt)

        # v projection: [2048, 384] natural layout, strided into v_s
        for kt in range(NKT):
            ps = proj_ps.tile([P, HB], f32, tag="proj")
            for k in range(KT):
                nc.tensor.matmul(
                    ps,
                    xv_t[k][:, kt * P : (kt + 1) * P],
                    wv_s[:, k, :],
                    start=(k == 0),
                    stop=(k == KT - 1),
                )
            nc.vector.tensor_copy(
                v_s[:, kt, :, 0:DH], ps.rearrange("p (h m) -> p h m", m=DH)
            )

        xn_t = [xnp.tile([P, NQ], bf16, tag="xn", name=f"xn{i}") for i in range(3)]

        for j4 in range(NCH):
            qsl = slice(j4 * QCH, (j4 + 1) * QCH)
            for p3 in range(3):
                rows = (slice(0, DH), slice(DH, 2 * DH))
                p_tiles = ([], [])
                # QK^T row-packed across the head pair + exp
                for ktp in range(NKT // 2):
                    lts = [lt_ps.tile([P, 2 * QCH], f32, tag="lt", name=f"lt{i}") for i in range(2)]
                    for u in range(2):
                        kt = 2 * ktp + u
                        for hh in range(2):
                            nc.tensor.matmul(
                                lts[hh][:, u * QCH : (u + 1) * QCH],
                                kT_t[p3][rows[hh], kt * P : (kt + 1) * P],
                                qT_t[p3][rows[hh], qsl],
                                start=True,
                                stop=True,
                            )
                    for hh in range(2):
                        pt = pp.tile([P, 2 * QCH], bf16, tag="p")
                        nc.scalar.activation(pt, lts[hh], EXP)
                        p_tiles[hh].append(pt)
                # AV with ones-augmented v: rows 0..63 = x, row 64 = denom
                for hh in range(2):
                    h = 2 * p3 + hh
                    xps = x_ps.tile([DH + 1, QCH], f32, tag="x")
                    for kt in range(NKT):
                        pt = p_tiles[hh][kt // 2][:, (kt % 2) * QCH : (kt % 2 + 1) * QCH]
                        nc.tensor.matmul(
                            xps,
                            v_s[:, kt, h, :],
                            pt,
                            start=(kt == 0),
                            stop=(kt == NKT - 1),
                        )
                    r = small.tile([1, QCH], f32, tag="r")
                    nc.vector.reciprocal(r, xps[DH : DH + 1, :])
                    rd = drp.tile([1, QCH], f32, tag="rd")
                    nc.sync.dma_start(out=rd, in_=r)
                    rb = small.tile([DH, QCH], f32, tag="rb")
                    nc.gpsimd.dma_start(out=rb, in_=rd.to_broadcast([DH, QCH]))
                    if hh == 0:
                        nc.vector.tensor_mul(xn_t[p3][0:DH, qsl], xps[0:DH, :], rb)
                    else:
                        tmp = small.tile([DH, QCH], bf16, tag="tmp")
                        nc.vector.tensor_mul(tmp, xps[0:DH, :], rb)
                        nc.sync.dma_start(out=xn_t[p3][DH : 2 * DH, qsl], in_=tmp)
            # output projection for this chunk
            for mt in range(C // P):
                ps = proj_ps.tile([P, QCH], f32, tag="proj")
                for k3 in range(HB // P):
                    nc.tensor.matmul(
                        ps,
                        wo_s[:, k3, mt * P : (mt + 1) * P],
                        xn_t[k3][:, qsl],
                        start=(k3 == 0),
                        stop=(k3 == HB // P - 1),
                    )
                o = ost.tile([P, QCH], f32, tag="o")
                nc.vector.tensor_copy(o, ps)
                nc.sync.dma_start(out=out_d[mt * P : (mt + 1) * P, qsl], in_=o)

    nc.finalize()
    return nc


def _get_prog():
    global _prog
    if _prog is None:
        _prog = _build()
    return _prog


def _shard_inputs(query, key, value, Wq, Wk, Wv, Wo):
    from ml_dtypes import bfloat16

    in_maps = []
    for core in range(8):
        b, hg = core // 2, core % 2
        sl = slice(hg * HB, (hg + 1) * HB)
        in_maps.append(
            {
                "xq": np.ascontiguousarray(query[b].T).astype(bfloat16),
                "xk": np.ascontiguousarray(key[b].T).astype(bfloat16),
                "xv": np.ascontiguousarray(value[b].T).astype(bfloat16),
                "wq": np.ascontiguousarray((Wq[sl, :] * SCALE).T).astype(bfloat16),
                "wk": np.ascontiguousarray(Wk[sl, :].T).astype(bfloat16),
                "wv": np.ascontiguousarray(Wv[sl, :].T).astype(bfloat16),
                "wo": np.ascontiguousarray(Wo[:, sl].T).astype(bfloat16),
            }
        )
    return in_maps


def kernel(query, key, value, Wq, Wk, Wv, Wo, bo):
    query, key, value = np.asarray(query), np.asarray(key), np.asarray(value)
    Wq, Wk, Wv, Wo = np.asarray(Wq), np.asarray(Wk), np.asarray(Wv), np.asarray(Wo)
    bo = np.asarray(bo).astype(np.float32)

    from concourse.bass_utils import run_bass_kernel_spmd

    nc = _get_prog()
    in_maps = _shard_inputs(query, key, value, Wq, Wk, Wv, Wo)
    res = run_bass_kernel_spmd(nc, in_maps, list(range(8))).results

    out = np.empty((B, NQ, C), np.float32)
    for b in range(B):
        acc = res[2 * b]["out"].astype(np.float32) + res[2 * b + 1]["out"].astype(
            np.float32
        )
        out[b] = acc.T + bo[None, :]
    return out



# revision 14
# speedup vs baseline: 1.2441x; 1.2441x over previous
"""Cross-attention kernel for 8 Trainium2 NeuronCores.

Sharding: 8 cores = 4 batches x 2 head-groups (6 heads each).
Per core (b, hg), with all activations pre-transposed on host:
  qT = (Wq_hg*scale).T' @ xqT   [384, 2048]   (weights column-split)
  kT = Wk_hg' @ xkT             [384, 2048]
  v  = xvT' @ Wv_hg.T           [2048, 384]  (+ ones column per head)
  per head h: lt = k_h qT_h     [2048k, 2048q] (logits transposed)
              p  = exp(lt)      (no max-subtraction: logits are O(1))
              [x; d] = [v_h|1].T @ p   -> x rows 0..63, denominators row 64
              xn = x * (1/d)    (partition-broadcast of 1/d)
  outT_partial = Wo_hg.T' @ xn  [768, 2048]
Host: out[b] = (partial[2b] + partial[2b+1]).T + bo.

All matmuls bf16 inputs with fp32 PSUM accumulation.

v2 (same math as baseline, denser pipeline):
- reciprocal_approx_fast (single custom-DVE op) replaces InstReciprocal
  (3.3us x24 = 80us in the baseline trace).
- gpsimd partition_broadcast replaces the DRAM-bounce broadcast DMA.
- Fine-grained emission: per (pair, chunk) unit, 8 "ktp blocks" of
  [QKT 4mm; 2 exps; fillers; AV of previous ktp 4mm]. The v/pair-1/2
  projections and the out-projections ride as fillers inside early
  units so the PE stream is dense and ACT (exp, ~214us total - the
  steady-state bottleneck) is never starved.
- Input DMAs ordered wq,wk,xq,xk first so projections start early.
"""

import sys

import numpy as np

for _p in ("/opt/trn_rl_repo",):
    if _p not in sys.path:
        sys.path.insert(0, _p)

B, NQ, NK, C = 4, 2048, 2048, 768
H, DH = 12, 64
HPC, HB = 6, 384  # heads per core, head-block width
P = 128
KT = C // P  # 6 contraction k-tiles for projections
QCH = 512  # query-chunk width
NCH = NQ // QCH  # 4 query chunks
NKT = NK // P  # 16 key tiles
SCALE = DH**-0.5  # folded into Wq on host (exactly 0.125)
VW = DH + 1  # v block width per head incl. ones column

_prog = None
DEBUG = False


def _build():
    from contextlib import ExitStack

    import concourse.bass as bass
    import concourse.tile as tile
    from concourse import library_config, mybir
    from concourse.bacc import Bacc

    f32 = mybir.dt.float32
    bf16 = mybir.dt.bfloat16
    EXP = mybir.ActivationFunctionType.Exp

    nc = Bacc()
    xq_d = nc.declare_dram_parameter("xq", [C, NQ], bf16, isOutput=False)
    xk_d = nc.declare_dram_parameter("xk", [C, NK], bf16, isOutput=False)
    xv_d = nc.declare_dram_parameter("xv", [C, NK], bf16, isOutput=False)
    wq_d = nc.declare_dram_parameter("wq", [C, HB], bf16, isOutput=False)
    wk_d = nc.declare_dram_parameter("wk", [C, HB], bf16, isOutput=False)
    wv_d = nc.declare_dram_parameter("wv", [C, HB], bf16, isOutput=False)
    wo_d = nc.declare_dram_parameter("wo", [HB, C], bf16, isOutput=False)
    out_d = nc.declare_dram_parameter("out", [C, NQ], f32, isOutput=True)
    if DEBUG:
        dbg_d = {
            "dbg_d": nc.declare_dram_parameter("dbg_d", [1, QCH], f32, isOutput=True),
            "dbg_r": nc.declare_dram_parameter("dbg_r", [1, QCH], f32, isOutput=True),
            "dbg_rb": nc.declare_dram_parameter("dbg_rb", [DH, QCH], f32, isOutput=True),
            "dbg_qT": nc.declare_dram_parameter("dbg_qT", [P, NQ], f32, isOutput=True),
            "dbg_xn": nc.declare_dram_parameter("dbg_xn", [P, NQ], f32, isOutput=True),
        }

    with tile.TileContext(nc) as tc, ExitStack() as ctx:
        const = ctx.enter_context(tc.tile_pool(name="const", bufs=1))
        xin = ctx.enter_context(tc.tile_pool(name="xin", bufs=1))
        qk = ctx.enter_context(tc.tile_pool(name="qk", bufs=1))
        pp = ctx.enter_context(tc.tile_pool(name="pp", bufs=6))
        xnp = ctx.enter_context(tc.tile_pool(name="xnp", bufs=1))
        small = ctx.enter_context(tc.tile_pool(name="small", bufs=4))
        ost = ctx.enter_context(tc.tile_pool(name="ost", bufs=3))
        proj_ps = ctx.enter_context(tc.tile_pool(name="proj_ps", bufs=2, space="PSUM"))
        lt_ps = ctx.enter_context(tc.tile_pool(name="lt_ps", bufs=2, space="PSUM"))
        x_ps = ctx.enter_context(tc.tile_pool(name="x_ps", bufs=2, space="PSUM"))

        # gpsimd ucode library for partition_broadcast
        nc.gpsimd.load_library(library_config.attn)

        # ---- input DMAs, priority order: q/k weights+activations first
        wq_s = const.tile([P, KT, HB], bf16, tag="wq")
        wk_s = const.tile([P, KT, HB], bf16, tag="wk")
        nc.sync.dma_start(out=wq_s, in_=wq_d.rearrange("(k p) m -> p k m", p=P))
        nc.sync.dma_start(out=wk_s, in_=wk_d.rearrange("(k p) m -> p k m", p=P))

        xq_t, xk_t, xv_t = [], [], []
        for name, dram, lst in (("xq", xq_d, xq_t), ("xk", xk_d, xk_t)):
            for k in range(KT):
                t = xin.tile([P, NQ], bf16, tag=f"{name}_{k}", name=f"{name}_{k}")
                nc.sync.dma_start(
                    out=t, in_=dram.rearrange("(k p) m -> p k m", p=P)[:, k, :]
                )
                lst.append(t)

        wv_s = const.tile([P, KT, HB], bf16, tag="wv")
        wo_s = const.tile([P, HB // P, C], bf16, tag="wo")
        nc.sync.dma_start(out=wv_s, in_=wv_d.rearrange("(k p) m -> p k m", p=P))
        nc.sync.dma_start(out=wo_s, in_=wo_d.rearrange("(k p) m -> p k m", p=P))
        for k in range(KT):
            t = xin.tile([P, NQ], bf16, tag=f"xv_{k}", name=f"xv_{k}")
            nc.sync.dma_start(
                out=t, in_=xv_d.rearrange("(k p) m -> p k m", p=P)[:, k, :]
            )
            xv_t.append(t)

        # v with a ones column per head: [128, kt, head, 65]
        v_s = const.tile([P, NKT, HPC, VW], bf16, tag="v")
        nc.vector.memset(v_s[:, :, :, DH : DH + 1], 1.0)

        qT_t = [qk.tile([P, NQ], bf16, tag=f"qT{i}", name=f"qT{i}") for i in range(3)]
        kT_t = [qk.tile([P, NQ], bf16, tag=f"kT{i}", name=f"kT{i}") for i in range(3)]
        xn_t = [xnp.tile([P, NQ], bf16, tag=f"xn{i}", name=f"xn{i}") for i in range(3)]

        # ---- small emitters used as PE fillers
        def projqk_block(w_s, src, dst, mt, j4):
            def f():
                ps = proj_ps.tile([P, QCH], f32, tag="proj", name="ps")
                for k in range(KT):
                    nc.tensor.matmul(
                        ps,
                        w_s[:, k, mt * P : (mt + 1) * P],
                        src[k][:, j4 * QCH : (j4 + 1) * QCH],
                        start=(k == 0),
                        stop=(k == KT - 1),
                    )
                nc.vector.tensor_copy(dst[:, j4 * QCH : (j4 + 1) * QCH], ps)

            return f

        def vproj_block(kt):
            def f():
                ps = proj_ps.tile([P, HB], f32, tag="proj", name="ps")
                for k in range(KT):
                    nc.tensor.matmul(
                        ps,
                        xv_t[k][:, kt * P : (kt + 1) * P],
                        wv_s[:, k, :],
                        start=(k == 0),
                        stop=(k == KT - 1),
                    )
                nc.vector.tensor_copy(
                    v_s[:, kt, :, 0:DH], ps.rearrange("p (h m) -> p h m", m=DH)
                )

            return f

        def outproj_block(j4, mt):
            def f():
                ps = proj_ps.tile([P, QCH], f32, tag="proj", name="ps")
                for k3 in range(HB // P):
                    nc.tensor.matmul(
                        ps,
                        wo_s[:, k3, mt * P : (mt + 1) * P],
                        xn_t[k3][:, j4 * QCH : (j4 + 1) * QCH],
                        start=(k3 == 0),
                        stop=(k3 == HB // P - 1),
                    )
                o = ost.tile([P, QCH], f32, tag="o", name="o")
                nc.vector.tensor_copy(o, ps)
                nc.sync.dma_start(
                    out=out_d[mt * P : (mt + 1) * P, j4 * QCH : (j4 + 1) * QCH], in_=o
                )

            return f

        # ---- attention unit = one (pair, chunk): 8 ktp blocks of
        # [QKT 4mm; 2 exps; fillers; AV(ktp-1) 4mm]. The last ktp's AV is
        # carried into the next unit's first block (or flushed at the end).
        rows = (slice(0, DH), slice(DH, 2 * DH))
        av_carry = [None]

        def av_block(p3, j4, ktp, p_pair, xd_pair):
            qsl = slice(j4 * QCH, (j4 + 1) * QCH)

            def f():
                for hh in range(2):
                    h = 2 * p3 + hh
                    for u in range(2):
                        kt = 2 * ktp + u
                        nc.tensor.matmul(
                            xd_pair[hh],
                            v_s[:, kt, h, :],
                            p_pair[hh][:, u * QCH : (u + 1) * QCH],
                            start=(kt == 0),
                            stop=(kt == NKT - 1),
                        )
                if ktp == NKT // 2 - 1:
                    # normalize both heads: r = 1/d, broadcast, multiply
                    for hh in range(2):
                        xd = xd_pair[hh]
                        # custom-DVE ops mishandle nonzero input base
                        # partition: copy the denominator row to partition 0
                        # first (plain tensor_copy handles the offset).
                        dcp = small.tile([1, QCH], f32, tag="dcp", name="dcp")
                        nc.vector.tensor_copy(dcp, xd[DH : DH + 1, :])
                        r = small.tile([1, QCH], f32, tag="r", name="r")
                        nc.vector.reciprocal_approx_fast(r, dcp)
                        rb = small.tile([DH, QCH], f32, tag="rb", name="rb")
                        nc.gpsimd.partition_broadcast(rb, r)
                        if DEBUG and p3 == 0 and j4 == 0 and hh == 0:
                            nc.sync.dma_start(out=dbg_d["dbg_d"][:, :], in_=dcp)
                            nc.sync.dma_start(out=dbg_d["dbg_r"][:, :], in_=r)
                            nc.sync.dma_start(out=dbg_d["dbg_rb"][:, :], in_=rb)
                        if hh == 0:
                            nc.vector.tensor_mul(
                                xn_t[p3][0:DH, qsl], xd[0:DH, :], rb
                            )
                        else:
                            tmp = small.tile([DH, QCH], bf16, tag="tmp", name="tmp")
                            nc.vector.tensor_mul(tmp, xd[0:DH, :], rb)
                            nc.sync.dma_start(
                                out=xn_t[p3][DH : 2 * DH, qsl], in_=tmp
                            )

            return f

        def attn_unit(p3, j4, fillers):
            """fillers: list of 8 lists of emitters, one list per ktp block,
            emitted BEFORE the trailing AV so region deps are in order."""
            qsl = slice(j4 * QCH, (j4 + 1) * QCH)
            xd_pair = [
                x_ps.tile([DH + 1, QCH], f32, tag="x", name=f"xd{hh}")
                for hh in range(2)
            ]
            for ktp in range(NKT // 2):
                lts = [
                    lt_ps.tile([P, 2 * QCH], f32, tag="lt", name=f"lt{hh}")
                    for hh in range(2)
                ]
                for u in range(2):
                    kt = 2 * ktp + u
                    for hh in range(2):
                        nc.tensor.matmul(
                            lts[hh][:, u * QCH : (u + 1) * QCH],
                            kT_t[p3][rows[hh], kt * P : (kt + 1) * P],
                            qT_t[p3][rows[hh], qsl],
                            start=True,
                            stop=True,
                        )
                p_pair = []
                for hh in range(2):
                    pt = pp.tile([P, 2 * QCH], bf16, tag="p", name="pt")
                    nc.scalar.activation(pt, lts[hh], EXP)
                    p_pair.append(pt)
                if av_carry[0] is not None:
                    av_carry[0]()
                av_carry[0] = av_block(p3, j4, ktp, p_pair, xd_pair)
                for f in fillers[ktp]:
                    f()

        def spread(emitters):
            """Distribute a list of emitters over 8 ktp blocks, in order."""
            out = [[] for _ in range(8)]
            for i, e in enumerate(emitters):
                out[min(i * 8 // max(len(emitters), 1), 7)].append(e)
            return out

        # ---- emission schedule
        # pair-0 q/k projections run first (PE starts once xq/xk arrive)
        for j4 in range(NCH):
            projqk_block(wq_s, xq_t, qT_t[0], 0, j4)()
        for j4 in range(NCH):
            projqk_block(wk_s, xk_t, kT_t[0], 0, j4)()

        # unit (p0,c0): fillers = v-projection (2 per block, emitted before
        # the trailing AV(ktp-1), which needs v tiles 2ktp-2, 2ktp-1) and
        # pair-1 q/k projections (1 per block).
        u0_fillers = []
        qkp1 = [
            projqk_block((wq_s, wk_s)[i % 2], (xq_t, xk_t)[i % 2],
                         (qT_t, kT_t)[i % 2][1], 1, i // 2)
            for i in range(8)
        ]
        for i in range(8):
            u0_fillers.append([vproj_block(2 * i), vproj_block(2 * i + 1), qkp1[i]])
        attn_unit(0, 0, u0_fillers)

        # unit (p1,c0): fillers = pair-2 q/k projections
        qkp2 = [
            projqk_block((wq_s, wk_s)[i % 2], (xq_t, xk_t)[i % 2],
                         (qT_t, kT_t)[i % 2][2], 2, i // 2)
            for i in range(8)
        ]
        attn_unit(1, 0, [[qkp2[i]] for i in range(8)])

        attn_unit(2, 0, [[] for _ in range(8)])

        # chunks 1..3: out-proj of previous chunk rides in the first unit
        for j4 in range(1, NCH):
            op = [outproj_block(j4 - 1, mt) for mt in range(C // P)]
            attn_unit(0, j4, spread(op))
            attn_unit(1, j4, [[] for _ in range(8)])
            attn_unit(2, j4, [[] for _ in range(8)])

        # flush: last AV+normalize, then final chunk's out-proj
        av_carry[0]()
        av_carry[0] = None
        for mt in range(C // P):
            outproj_block(NCH - 1, mt)()

        if DEBUG:
            for nm, t in (("dbg_qT", qT_t[0]), ("dbg_xn", xn_t[0])):
                dc = ost.tile([P, NQ], f32, tag="dbgc", name="dbgc", bufs=1)
                nc.vector.tensor_copy(dc, t)
                nc.sync.dma_start(out=dbg_d[nm][:, :], in_=dc)

    nc.finalize()
    return nc


def _get_prog():
    global _prog
    if _prog is None:
        _prog = _build()
    return _prog


def _shard_inputs(query, key, value, Wq, Wk, Wv, Wo):
    from ml_dtypes import bfloat16

    in_maps = []
    for core in range(8):
        b, hg = core // 2, core % 2
        sl = slice(hg * HB, (hg + 1) * HB)
        in_maps.append(
            {
                "xq": np.ascontiguousarray(query[b].T).astype(bfloat16),
                "xk": np.ascontiguousarray(key[b].T).astype(bfloat16),
                "xv": np.ascontiguousarray(value[b].T).astype(bfloat16),
                "wq": np.ascontiguousarray((Wq[sl, :] * SCALE).T).astype(bfloat16),
                "wk": np.ascontiguousarray(Wk[sl, :].T).astype(bfloat16),
                "wv": np.ascontiguousarray(Wv[sl, :].T).astype(bfloat16),
                "wo": np.ascontiguousarray(Wo[:, sl].T).astype(bfloat16),
            }
        )
    return in_maps


def kernel(query, key, value, Wq, Wk, Wv, Wo, bo):
    query, key, value = np.asarray(query), np.asarray(key), np.asarray(value)
    Wq, Wk, Wv, Wo = np.asarray(Wq), np.asarray(Wk), np.asarray(Wv), np.asarray(Wo)
    bo = np.asarray(bo).astype(np.float32)

    from concourse.bass_utils import run_bass_kernel_spmd

    nc = _get_prog()
    in_maps = _shard_inputs(query, key, value, Wq, Wk, Wv, Wo)
    res = run_bass_kernel_spmd(nc, in_maps, list(range(8))).results

    out = np.empty((B, NQ, C), np.float32)
    for b in range(B):
        acc = res[2 * b]["out"].astype(np.float32) + res[2 * b + 1]["out"].astype(
            np.float32
        )
        out[b] = acc.T + bo[None, :]
    return out


# revision 16
# speedup vs baseline: 1.3535x; 1.0880x over previous
"""Cross-attention kernel for 8 Trainium2 NeuronCores.

Sharding: 8 cores = 4 batches x 2 head-groups (6 heads each).
Per core (b, hg), with all activations pre-transposed on host:
  qT = (Wq_hg*scale).T' @ xqT   [384, 2048]   (weights column-split)
  kT = Wk_hg' @ xkT             [384, 2048]
  v  = xvT' @ Wv_hg.T           [2048, 384]  (+ ones column per head)
  per head h: lt = k_h qT_h     [2048k, 2048q] (logits transposed)
              p  = exp(lt)      (no max-subtraction: logits are O(1))
              [x; d] = [v_h|1].T @ p   -> x rows 0..63, denominators row 64
              xn = x * (1/d)    (partition-broadcast of 1/d)
  outT_partial = Wo_hg.T' @ xn  [768, 2048]
Host: out[b] = (partial[2b] + partial[2b+1]).T + bo.

All matmuls bf16 inputs with fp32 PSUM accumulation.

v2 (same math as baseline, denser pipeline):
- reciprocal_approx_fast (single custom-DVE op) replaces InstReciprocal
  (3.3us x24 = 80us in the baseline trace).
- gpsimd partition_broadcast replaces the DRAM-bounce broadcast DMA.
- Fine-grained emission: per (pair, chunk) unit, 8 "ktp blocks" of
  [QKT 4mm; 2 exps; fillers; AV of previous ktp 4mm]. The v/pair-1/2
  projections and the out-projections ride as fillers inside early
  units so the PE stream is dense and ACT (exp, ~214us total - the
  steady-state bottleneck) is never starved.
- Input DMAs ordered wq,wk,xq,xk first so projections start early.
"""

import sys

import numpy as np

for _p in ("/opt/trn_rl_repo",):
    if _p not in sys.path:
        sys.path.insert(0, _p)

B, NQ, NK, C = 4, 2048, 2048, 768
H, DH = 12, 64
HPC, HB = 6, 384  # heads per core, head-block width
P = 128
KT = C // P  # 6 contraction k-tiles for projections
QCH = 512  # query-chunk width
NCH = NQ // QCH  # 4 query chunks
NKT = NK // P  # 16 key tiles
SCALE = DH**-0.5  # folded into Wq on host (exactly 0.125)
VW = DH + 1  # v block width per head incl. ones column

_prog = None
DEBUG = False


def _build():
    from contextlib import ExitStack

    import concourse.bass as bass
    import concourse.tile as tile
    from concourse import library_config, mybir
    from concourse.bacc import Bacc

    f32 = mybir.dt.float32
    bf16 = mybir.dt.bfloat16
    EXP = mybir.ActivationFunctionType.Exp

    nc = Bacc()
    xq_d = nc.declare_dram_parameter("xq", [C, NQ], bf16, isOutput=False)
    xk_d = nc.declare_dram_parameter("xk", [C, NK], bf16, isOutput=False)
    xv_d = nc.declare_dram_parameter("xv", [C, NK], bf16, isOutput=False)
    wq_d = nc.declare_dram_parameter("wq", [C, HB], bf16, isOutput=False)
    wk_d = nc.declare_dram_parameter("wk", [C, HB], bf16, isOutput=False)
    wv_d = nc.declare_dram_parameter("wv", [C, HB], bf16, isOutput=False)
    wo_d = nc.declare_dram_parameter("wo", [HB, C], bf16, isOutput=False)
    out_d = nc.declare_dram_parameter("out", [C, NQ], f32, isOutput=True)
    if DEBUG:
        dbg_d = {
            "dbg_d": nc.declare_dram_parameter("dbg_d", [1, QCH], f32, isOutput=True),
            "dbg_r": nc.declare_dram_parameter("dbg_r", [1, QCH], f32, isOutput=True),
            "dbg_rb": nc.declare_dram_parameter("dbg_rb", [DH, QCH], f32, isOutput=True),
            "dbg_qT": nc.declare_dram_parameter("dbg_qT", [P, NQ], f32, isOutput=True),
            "dbg_xn": nc.declare_dram_parameter("dbg_xn", [P, NQ], f32, isOutput=True),
        }

    with tile.TileContext(nc) as tc, ExitStack() as ctx:
        const = ctx.enter_context(tc.tile_pool(name="const", bufs=1))
        xin = ctx.enter_context(tc.tile_pool(name="xin", bufs=1))
        qk = ctx.enter_context(tc.tile_pool(name="qk", bufs=1))
        pp = ctx.enter_context(tc.tile_pool(name="pp", bufs=6))
        xnp = ctx.enter_context(tc.tile_pool(name="xnp", bufs=1))
        small = ctx.enter_context(tc.tile_pool(name="small", bufs=4))
        ost = ctx.enter_context(tc.tile_pool(name="ost", bufs=3))
        proj_ps = ctx.enter_context(tc.tile_pool(name="proj_ps", bufs=2, space="PSUM"))
        lt_ps = ctx.enter_context(tc.tile_pool(name="lt_ps", bufs=2, space="PSUM"))
        x_ps = ctx.enter_context(tc.tile_pool(name="x_ps", bufs=2, space="PSUM"))

        # gpsimd ucode library for partition_broadcast
        nc.gpsimd.load_library(library_config.attn)

        # ---- input DMAs, priority order: q/k weights+activations first
        wq_s = const.tile([P, KT, HB], bf16, tag="wq")
        wk_s = const.tile([P, KT, HB], bf16, tag="wk")
        nc.sync.dma_start(out=wq_s, in_=wq_d.rearrange("(k p) m -> p k m", p=P))
        nc.sync.dma_start(out=wk_s, in_=wk_d.rearrange("(k p) m -> p k m", p=P))

        xq_t, xk_t, xv_t = [], [], []
        for name, dram, lst in (("xq", xq_d, xq_t), ("xk", xk_d, xk_t)):
            for k in range(KT):
                t = xin.tile([P, NQ], bf16, tag=f"{name}_{k}", name=f"{name}_{k}")
                nc.sync.dma_start(
                    out=t, in_=dram.rearrange("(k p) m -> p k m", p=P)[:, k, :]
                )
                lst.append(t)

        wv_s = const.tile([P, KT, HB], bf16, tag="wv")
        wo_s = const.tile([P, HB // P, C], bf16, tag="wo")
        nc.sync.dma_start(out=wv_s, in_=wv_d.rearrange("(k p) m -> p k m", p=P))
        nc.sync.dma_start(out=wo_s, in_=wo_d.rearrange("(k p) m -> p k m", p=P))
        for k in range(KT):
            t = xin.tile([P, NQ], bf16, tag=f"xv_{k}", name=f"xv_{k}")
            nc.sync.dma_start(
                out=t, in_=xv_d.rearrange("(k p) m -> p k m", p=P)[:, k, :]
            )
            xv_t.append(t)

        # v with a ones column per head: [128, kt, head, 65]
        v_s = const.tile([P, NKT, HPC, VW], bf16, tag="v")
        nc.vector.memset(v_s[:, :, :, DH : DH + 1], 1.0)

        qT_t = [qk.tile([P, NQ], bf16, tag=f"qT{i}", name=f"qT{i}") for i in range(3)]
        kT_t = [qk.tile([P, NQ], bf16, tag=f"kT{i}", name=f"kT{i}") for i in range(3)]
        xn_t = [xnp.tile([P, NQ], bf16, tag=f"xn{i}", name=f"xn{i}") for i in range(3)]

        # ---- small emitters used as PE fillers
        def projqk_block(w_s, src, dst, mt, j4):
            def f():
                ps = proj_ps.tile([P, QCH], f32, tag="proj", name="ps")
                for k in range(KT):
                    nc.tensor.matmul(
                        ps,
                        w_s[:, k, mt * P : (mt + 1) * P],
                        src[k][:, j4 * QCH : (j4 + 1) * QCH],
                        start=(k == 0),
                        stop=(k == KT - 1),
                    )
                nc.vector.tensor_copy(dst[:, j4 * QCH : (j4 + 1) * QCH], ps)

            return f

        def vproj_block(kt):
            def f():
                ps = proj_ps.tile([P, HB], f32, tag="proj", name="ps")
                for k in range(KT):
                    nc.tensor.matmul(
                        ps,
                        xv_t[k][:, kt * P : (kt + 1) * P],
                        wv_s[:, k, :],
                        start=(k == 0),
                        stop=(k == KT - 1),
                    )
                nc.vector.tensor_copy(
                    v_s[:, kt, :, 0:DH], ps.rearrange("p (h m) -> p h m", m=DH)
                )

            return f

        def outproj_block(j4, mt):
            def f():
                ps = proj_ps.tile([P, QCH], f32, tag="proj", name="ps")
                for k3 in range(HB // P):
                    nc.tensor.matmul(
                        ps,
                        wo_s[:, k3, mt * P : (mt + 1) * P],
                        xn_t[k3][:, j4 * QCH : (j4 + 1) * QCH],
                        start=(k3 == 0),
                        stop=(k3 == HB // P - 1),
                    )
                o = ost.tile([P, QCH], f32, tag="o", name="o")
                nc.vector.tensor_copy(o, ps)
                nc.sync.dma_start(
                    out=out_d[mt * P : (mt + 1) * P, j4 * QCH : (j4 + 1) * QCH], in_=o
                )

            return f

        # ---- attention unit = one (pair, chunk): 8 ktp blocks of
        # [QKT 4mm; 2 exps; fillers; AV(ktp-1) 4mm]. The last ktp's AV is
        # carried into the next unit's first block (or flushed at the end).
        rows = (slice(0, DH), slice(DH, 2 * DH))
        av_carry = [None]

        def av_block(p3, j4, ktp, p_pair, xd_pair):
            qsl = slice(j4 * QCH, (j4 + 1) * QCH)

            def f():
                for hh in range(2):
                    h = 2 * p3 + hh
                    for u in range(2):
                        kt = 2 * ktp + u
                        nc.tensor.matmul(
                            xd_pair[hh],
                            v_s[:, kt, h, :],
                            p_pair[hh][:, u * QCH : (u + 1) * QCH],
                            start=(kt == 0),
                            stop=(kt == NKT - 1),
                        )
                if ktp == NKT // 2 - 1:
                    # normalize both heads: r = 1/d, broadcast, multiply
                    for hh in range(2):
                        xd = xd_pair[hh]
                        # custom-DVE ops mishandle nonzero input base
                        # partition: copy the denominator row to partition 0
                        # first (plain tensor_copy handles the offset).
                        dcp = small.tile([1, QCH], f32, tag="dcp", name="dcp")
                        nc.vector.tensor_copy(dcp, xd[DH : DH + 1, :])
                        r = small.tile([1, QCH], f32, tag="r", name="r")
                        nc.vector.reciprocal_approx_fast(r, dcp)
                        rb = small.tile([DH, QCH], f32, tag="rb", name="rb")
                        nc.gpsimd.partition_broadcast(rb, r)
                        if DEBUG and p3 == 0 and j4 == 0 and hh == 0:
                            nc.sync.dma_start(out=dbg_d["dbg_d"][:, :], in_=dcp)
                            nc.sync.dma_start(out=dbg_d["dbg_r"][:, :], in_=r)
                            nc.sync.dma_start(out=dbg_d["dbg_rb"][:, :], in_=rb)
                        if hh == 0:
                            nc.vector.tensor_mul(
                                xn_t[p3][0:DH, qsl], xd[0:DH, :], rb
                            )
                        else:
                            tmp = small.tile([DH, QCH], bf16, tag="tmp", name="tmp")
                            nc.vector.tensor_mul(tmp, xd[0:DH, :], rb)
                            nc.sync.dma_start(
                                out=xn_t[p3][DH : 2 * DH, qsl], in_=tmp
                            )

            return f

        def attn_unit(p3, j4, fillers):
            """fillers: list of 8 lists of emitters, one list per ktp block,
            emitted BEFORE the trailing AV so region deps are in order."""
            qsl = slice(j4 * QCH, (j4 + 1) * QCH)
            xd_pair = [
                x_ps.tile([DH + 1, QCH], f32, tag="x", name=f"xd{hh}")
                for hh in range(2)
            ]
            for ktp in range(NKT // 2):
                lts = [
                    lt_ps.tile([P, 2 * QCH], f32, tag="lt", name=f"lt{hh}")
                    for hh in range(2)
                ]
                for u in range(2):
                    kt = 2 * ktp + u
                    for hh in range(2):
                        nc.tensor.matmul(
                            lts[hh][:, u * QCH : (u + 1) * QCH],
                            kT_t[p3][rows[hh], kt * P : (kt + 1) * P],
                            qT_t[p3][rows[hh], qsl],
                            start=True,
                            stop=True,
                        )
                p_pair = []
                for hh in range(2):
                    pt = pp.tile([P, 2 * QCH], bf16, tag="p", name="pt")
                    nc.scalar.activation(pt, lts[hh], EXP)
                    p_pair.append(pt)
                if av_carry[0] is not None:
                    av_carry[0]()
                av_carry[0] = av_block(p3, j4, ktp, p_pair, xd_pair)
                for f in fillers[ktp]:
                    f()

        def spread(emitters):
            """Distribute a list of emitters over 8 ktp blocks, in order."""
            out = [[] for _ in range(8)]
            for i, e in enumerate(emitters):
                out[min(i * 8 // max(len(emitters), 1), 7)].append(e)
            return out

        # ---- emission schedule. Region-level deps let QKT(p,c,ktp) start
        # once qT[p] chunk c and kT[p] key-block ktp//2 are projected, so
        # only qp0(j0)+kp0(j0) run before the first attention unit; every
        # other projection block rides as a filler.
        def qp(p3, j4):
            return projqk_block(wq_s, xq_t, qT_t[p3], p3, j4)

        def kp(p3, j4):
            return projqk_block(wk_s, xk_t, kT_t[p3], p3, j4)

        qp(0, 0)()
        kp(0, 0)()

        # unit (p0,c0): v-projection (2/block, before the trailing AV(ktp-1)
        # which needs v tiles 2ktp-2,2ktp-1), kp0 key-blocks just in time
        # (QKT ktp needs kp0(ktp//2)), and pair-1 j0 projections at the end.
        u0 = [[] for _ in range(8)]
        for i in range(8):
            u0[i] += [vproj_block(2 * i), vproj_block(2 * i + 1)]
        u0[1].append(kp(0, 1))
        u0[3].append(kp(0, 2))
        u0[5].append(kp(0, 3))
        u0[6].append(qp(1, 0))
        u0[7].append(kp(1, 0))
        attn_unit(0, 0, u0)

        # unit (p1,c0): rest of kp1 just in time, pair-2 j0 at the end
        u1 = [[] for _ in range(8)]
        u1[1].append(kp(1, 1))
        u1[3].append(kp(1, 2))
        u1[5].append(kp(1, 3))
        u1[6].append(qp(2, 0))
        u1[7].append(kp(2, 0))
        attn_unit(1, 0, u1)

        # unit (p2,c0): rest of kp2, plus q projections for chunk 1
        u2 = [[] for _ in range(8)]
        u2[1].append(kp(2, 1))
        u2[3].append(kp(2, 2))
        u2[5].append(kp(2, 3))
        u2[6].append(qp(0, 1))
        u2[7].append(qp(1, 1))
        attn_unit(2, 0, u2)

        # chunks 1..3: out-proj of the previous chunk rides in the first
        # unit (blocks 2..7); q projections for later chunks trickle in
        # one unit ahead of their consumer.
        for j4 in range(1, NCH):
            ua = [[] for _ in range(8)]
            for mt in range(C // P):
                ua[2 + mt].append(outproj_block(j4 - 1, mt))
            attn_unit(0, j4, ua)
            ub = [[] for _ in range(8)]
            ub[6].append(qp(2, j4))
            attn_unit(1, j4, ub)
            uc = [[] for _ in range(8)]
            if j4 < NCH - 1:
                uc[6].append(qp(0, j4 + 1))
                uc[7].append(qp(1, j4 + 1))
            attn_unit(2, j4, uc)

        # flush: last AV+normalize, then final chunk's out-proj
        av_carry[0]()
        av_carry[0] = None
        for mt in range(C // P):
            outproj_block(NCH - 1, mt)()

        if DEBUG:
            for nm, t in (("dbg_qT", qT_t[0]), ("dbg_xn", xn_t[0])):
                dc = ost.tile([P, NQ], f32, tag="dbgc", name="dbgc", bufs=1)
                nc.vector.tensor_copy(dc, t)
                nc.sync.dma_start(out=dbg_d[nm][:, :], in_=dc)

    nc.finalize()
    return nc


def _get_prog():
    global _prog
    if _prog is None:
        _prog = _build()
    return _prog


def _shard_inputs(query, key, value, Wq, Wk, Wv, Wo):
    from ml_dtypes import bfloat16

    in_maps = []
    for core in range(8):
        b, hg = core // 2, core % 2
        sl = slice(hg * HB, (hg + 1) * HB)
        in_maps.append(
            {
                "xq": np.ascontiguousarray(query[b].T).astype(bfloat16),
                "xk": np.ascontiguousarray(key[b].T).astype(bfloat16),
                "xv": np.ascontiguousarray(value[b].T).astype(bfloat16),
                "wq": np.ascontiguousarray((Wq[sl, :] * SCALE).T).astype(bfloat16),
                "wk": np.ascontiguousarray(Wk[sl, :].T).astype(bfloat16),
                "wv": np.ascontiguousarray(Wv[sl, :].T).astype(bfloat16),
                "wo": np.ascontiguousarray(Wo[:, sl].T).astype(bfloat16),
            }
        )
    return in_maps


def kernel(query, key, value, Wq, Wk, Wv, Wo, bo):
    query, key, value = np.asarray(query), np.asarray(key), np.asarray(value)
    Wq, Wk, Wv, Wo = np.asarray(Wq), np.asarray(Wk), np.asarray(Wv), np.asarray(Wo)
    bo = np.asarray(bo).astype(np.float32)

    from concourse.bass_utils import run_bass_kernel_spmd

    nc = _get_prog()
    in_maps = _shard_inputs(query, key, value, Wq, Wk, Wv, Wo)
    res = run_bass_kernel_spmd(nc, in_maps, list(range(8))).results

    out = np.empty((B, NQ, C), np.float32)
    for b in range(B):
        acc = res[2 * b]["out"].astype(np.float32) + res[2 * b + 1]["out"].astype(
            np.float32
        )
        out[b] = acc.T + bo[None, :]
    return out


# revision 20
# speedup vs baseline: 1.3881x; 1.0256x over previous
"""Cross-attention kernel for 8 Trainium2 NeuronCores.

Sharding: 8 cores = 4 batches x 2 head-groups (6 heads each).
Per core (b, hg), with all activations pre-transposed on host:
  qT = (Wq_hg*scale).T' @ xqT   [384, 2048]   (weights column-split)
  kT = Wk_hg' @ xkT             [384, 2048]
  v  = xvT' @ Wv_hg.T           [2048, 384]  (+ ones column per head)
  per head h: lt = k_h qT_h     [2048k, 2048q] (logits transposed)
              p  = exp(lt)      (no max-subtraction: logits are O(1))
              [x; d] = [v_h|1].T @ p   -> x rows 0..63, denominators row 64
              xn = x * (1/d)    (partition-broadcast of 1/d)
  outT_partial = Wo_hg.T' @ xn  [768, 2048]
Host: out[b] = (partial[2b] + partial[2b+1]).T + bo.

All matmuls bf16 inputs with fp32 PSUM accumulation.

v2 (same math as baseline, denser pipeline):
- reciprocal_approx_fast (single custom-DVE op) replaces InstReciprocal
  (3.3us x24 = 80us in the baseline trace).
- gpsimd partition_broadcast replaces the DRAM-bounce broadcast DMA.
- Fine-grained emission: per (pair, chunk) unit, 8 "ktp blocks" of
  [QKT 4mm; 2 exps; fillers; AV of previous ktp 4mm]. The v/pair-1/2
  projections and the out-projections ride as fillers inside early
  units so the PE stream is dense and ACT (exp, ~214us total - the
  steady-state bottleneck) is never starved.
- Input DMAs ordered wq,wk,xq,xk first so projections start early.
"""

import sys

import numpy as np

for _p in ("/opt/trn_rl_repo",):
    if _p not in sys.path:
        sys.path.insert(0, _p)

B, NQ, NK, C = 4, 2048, 2048, 768
H, DH = 12, 64
HPC, HB = 6, 384  # heads per core, head-block width
P = 128
KT = C // P  # 6 contraction k-tiles for projections
QCH = 512  # query-chunk width
NCH = NQ // QCH  # 4 query chunks
NKT = NK // P  # 16 key tiles
SCALE = DH**-0.5  # folded into Wq on host (exactly 0.125)
VW = DH + 1  # v block width per head incl. ones column

_prog = None
DEBUG = False


def _build():
    from contextlib import ExitStack

    import concourse.bass as bass
    import concourse.tile as tile
    from concourse import library_config, mybir
    from concourse.bacc import Bacc

    f32 = mybir.dt.float32
    bf16 = mybir.dt.bfloat16
    EXP = mybir.ActivationFunctionType.Exp

    nc = Bacc()
    xq_d = nc.declare_dram_parameter("xq", [C, NQ], bf16, isOutput=False)
    xk_d = nc.declare_dram_parameter("xk", [C, NK], bf16, isOutput=False)
    xv_d = nc.declare_dram_parameter("xv", [C, NK], bf16, isOutput=False)
    wq_d = nc.declare_dram_parameter("wq", [C, HB], bf16, isOutput=False)
    wk_d = nc.declare_dram_parameter("wk", [C, HB], bf16, isOutput=False)
    wv_d = nc.declare_dram_parameter("wv", [C, HB], bf16, isOutput=False)
    wo_d = nc.declare_dram_parameter("wo", [HB, C], bf16, isOutput=False)
    out_d = nc.declare_dram_parameter("out", [C, NQ], f32, isOutput=True)
    if DEBUG:
        dbg_d = {
            "dbg_d": nc.declare_dram_parameter("dbg_d", [1, QCH], f32, isOutput=True),
            "dbg_r": nc.declare_dram_parameter("dbg_r", [1, QCH], f32, isOutput=True),
            "dbg_rb": nc.declare_dram_parameter("dbg_rb", [DH, QCH], f32, isOutput=True),
            "dbg_qT": nc.declare_dram_parameter("dbg_qT", [P, NQ], f32, isOutput=True),
            "dbg_xn": nc.declare_dram_parameter("dbg_xn", [P, NQ], f32, isOutput=True),
        }

    with tile.TileContext(nc) as tc, ExitStack() as ctx:
        const = ctx.enter_context(tc.tile_pool(name="const", bufs=1))
        xin = ctx.enter_context(tc.tile_pool(name="xin", bufs=1))
        qk = ctx.enter_context(tc.tile_pool(name="qk", bufs=1))
        pp = ctx.enter_context(tc.tile_pool(name="pp", bufs=18))
        xnp = ctx.enter_context(tc.tile_pool(name="xnp", bufs=1))
        small = ctx.enter_context(tc.tile_pool(name="small", bufs=2))
        ost = ctx.enter_context(tc.tile_pool(name="ost", bufs=3))
        proj_ps = ctx.enter_context(tc.tile_pool(name="proj_ps", bufs=2, space="PSUM"))
        lt_ps = ctx.enter_context(tc.tile_pool(name="lt_ps", bufs=2, space="PSUM"))
        x_ps = ctx.enter_context(tc.tile_pool(name="x_ps", bufs=2, space="PSUM"))

        # gpsimd ucode library for partition_broadcast
        nc.gpsimd.load_library(library_config.attn)

        # ---- input DMAs, priority order: q/k weights+activations first
        wq_s = const.tile([P, KT, HB], bf16, tag="wq")
        wk_s = const.tile([P, KT, HB], bf16, tag="wk")
        nc.sync.dma_start(out=wq_s, in_=wq_d.rearrange("(k p) m -> p k m", p=P))
        nc.sync.dma_start(out=wk_s, in_=wk_d.rearrange("(k p) m -> p k m", p=P))

        xq_t, xk_t, xv_t = [], [], []
        for name, dram, lst in (("xq", xq_d, xq_t), ("xk", xk_d, xk_t)):
            for k in range(KT):
                t = xin.tile([P, NQ], bf16, tag=f"{name}_{k}", name=f"{name}_{k}")
                nc.sync.dma_start(
                    out=t, in_=dram.rearrange("(k p) m -> p k m", p=P)[:, k, :]
                )
                lst.append(t)

        wv_s = const.tile([P, KT, HB], bf16, tag="wv")
        wo_s = const.tile([P, HB // P, C], bf16, tag="wo")
        nc.sync.dma_start(out=wv_s, in_=wv_d.rearrange("(k p) m -> p k m", p=P))
        nc.sync.dma_start(out=wo_s, in_=wo_d.rearrange("(k p) m -> p k m", p=P))
        for k in range(KT):
            t = xin.tile([P, NQ], bf16, tag=f"xv_{k}", name=f"xv_{k}")
            nc.sync.dma_start(
                out=t, in_=xv_d.rearrange("(k p) m -> p k m", p=P)[:, k, :]
            )
            xv_t.append(t)

        # v with a ones column per head: [128, kt, head, 65]
        v_s = const.tile([P, NKT, HPC, VW], bf16, tag="v")
        nc.vector.memset(v_s[:, :, :, DH : DH + 1], 1.0)

        qT_t = [qk.tile([P, NQ], bf16, tag=f"qT{i}", name=f"qT{i}") for i in range(3)]
        kT_t = [qk.tile([P, NQ], bf16, tag=f"kT{i}", name=f"kT{i}") for i in range(3)]
        xn_t = [xnp.tile([P, NQ], bf16, tag=f"xn{i}", name=f"xn{i}") for i in range(3)]

        # ---- small emitters used as PE fillers
        def projqk_block(w_s, src, dst, mt, j4):
            def f():
                ps = proj_ps.tile([P, QCH], f32, tag="proj", name="ps")
                for k in range(KT):
                    nc.tensor.matmul(
                        ps,
                        w_s[:, k, mt * P : (mt + 1) * P],
                        src[k][:, j4 * QCH : (j4 + 1) * QCH],
                        start=(k == 0),
                        stop=(k == KT - 1),
                    )
                nc.vector.tensor_copy(dst[:, j4 * QCH : (j4 + 1) * QCH], ps)

            return f

        def vproj_block(kt):
            def f():
                ps = proj_ps.tile([P, HB], f32, tag="proj", name="ps")
                for k in range(KT):
                    nc.tensor.matmul(
                        ps,
                        xv_t[k][:, kt * P : (kt + 1) * P],
                        wv_s[:, k, :],
                        start=(k == 0),
                        stop=(k == KT - 1),
                    )
                nc.vector.tensor_copy(
                    v_s[:, kt, :, 0:DH], ps.rearrange("p (h m) -> p h m", m=DH)
                )

            return f

        def outproj_block(j4, mt):
            def f():
                ps = proj_ps.tile([P, QCH], f32, tag="proj", name="ps")
                for k3 in range(HB // P):
                    nc.tensor.matmul(
                        ps,
                        wo_s[:, k3, mt * P : (mt + 1) * P],
                        xn_t[k3][:, j4 * QCH : (j4 + 1) * QCH],
                        start=(k3 == 0),
                        stop=(k3 == HB // P - 1),
                    )
                o = ost.tile([P, QCH], f32, tag="o", name="o")
                nc.vector.tensor_copy(o, ps)
                nc.sync.dma_start(
                    out=out_d[mt * P : (mt + 1) * P, j4 * QCH : (j4 + 1) * QCH], in_=o
                )

            return f

        # ---- attention unit = one (pair, chunk): 8 ktp blocks of
        # [QKT 4mm; 2 exps; AV block of the PREVIOUS unit; fillers].
        # The whole AV of unit u rides inside unit u+1 (uniform unit lag),
        # so AVs never gate on in-flight exps and xd PSUM slots alternate
        # between non-adjacent units.
        rows = (slice(0, DH), slice(DH, 2 * DH))
        av_carry = [[]]  # 8 pending AV-block emitters from the previous unit

        def av_block(p3, j4, ktp, p_pair, xd_pair):
            qsl = slice(j4 * QCH, (j4 + 1) * QCH)

            def f():
                for hh in range(2):
                    h = 2 * p3 + hh
                    for u in range(2):
                        kt = 2 * ktp + u
                        nc.tensor.matmul(
                            xd_pair[hh],
                            v_s[:, kt, h, :],
                            p_pair[hh][:, u * QCH : (u + 1) * QCH],
                            start=(kt == 0),
                            stop=(kt == NKT - 1),
                        )
                if ktp == NKT // 2 - 1:
                    # normalize both heads: r = 1/d, broadcast, multiply
                    for hh in range(2):
                        xd = xd_pair[hh]
                        # custom-DVE ops mishandle nonzero input base
                        # partition: copy the denominator row to partition 0
                        # first (plain tensor_copy handles the offset).
                        dcp = small.tile([1, QCH], f32, tag="dcp", name="dcp")
                        nc.vector.tensor_copy(dcp, xd[DH : DH + 1, :])
                        r = small.tile([1, QCH], f32, tag="r", name="r")
                        nc.vector.reciprocal_approx_fast(r, dcp)
                        rb = small.tile([DH, QCH], f32, tag="rb", name="rb")
                        nc.gpsimd.partition_broadcast(rb, r)
                        if hh == 0:
                            nc.vector.tensor_mul(
                                xn_t[p3][0:DH, qsl], xd[0:DH, :], rb
                            )
                        else:
                            tmp = small.tile([DH, QCH], bf16, tag="tmp", name="tmp")
                            nc.vector.tensor_mul(tmp, xd[0:DH, :], rb)
                            nc.sync.dma_start(
                                out=xn_t[p3][DH : 2 * DH, qsl], in_=tmp
                            )

            return f

        def attn_unit(p3, j4, fillers):
            """fillers: 8 lists of emitters, one per ktp block, emitted
            after the previous unit's AV block."""
            qsl = slice(j4 * QCH, (j4 + 1) * QCH)
            xd_pair = [
                x_ps.tile([DH + 1, QCH], f32, tag="x", name=f"xd{hh}")
                for hh in range(2)
            ]
            prev_av = av_carry[0]
            my_av = []
            for ktp in range(NKT // 2):
                lts = [
                    lt_ps.tile([P, 2 * QCH], f32, tag="lt", name=f"lt{hh}")
                    for hh in range(2)
                ]
                for u in range(2):
                    kt = 2 * ktp + u
                    for hh in range(2):
                        nc.tensor.matmul(
                            lts[hh][:, u * QCH : (u + 1) * QCH],
                            kT_t[p3][rows[hh], kt * P : (kt + 1) * P],
                            qT_t[p3][rows[hh], qsl],
                            start=True,
                            stop=True,
                        )
                p_pair = []
                for hh in range(2):
                    pt = pp.tile([P, 2 * QCH], bf16, tag="p", name="pt")
                    nc.scalar.activation(pt, lts[hh], EXP)
                    p_pair.append(pt)
                # previous unit's AVs on blocks 0..6 (two on block 0) so its
                # normalize completes well before the next unit reuses xd
                if ktp == 0:
                    for g in prev_av[0:2]:
                        g()
                elif ktp <= 6 and ktp + 1 < len(prev_av):
                    prev_av[ktp + 1]()
                my_av.append(av_block(p3, j4, ktp, p_pair, xd_pair))
                for f in fillers[ktp]:
                    f()
            av_carry[0] = my_av

        def flush_av():
            for f in av_carry[0]:
                f()
            av_carry[0] = []

        def spread(emitters):
            """Distribute a list of emitters over 8 ktp blocks, in order."""
            out = [[] for _ in range(8)]
            for i, e in enumerate(emitters):
                out[min(i * 8 // max(len(emitters), 1), 7)].append(e)
            return out

        # ---- emission schedule. Region-level deps let QKT(p,c,ktp) start
        # once qT[p] chunk c and kT[p] key-block ktp//2 are projected, so
        # only qp0(j0)+kp0(j0) run before the first attention unit; every
        # other projection block rides as a filler.
        def qp(p3, j4):
            return projqk_block(wq_s, xq_t, qT_t[p3], p3, j4)

        def kp(p3, j4):
            return projqk_block(wk_s, xk_t, kT_t[p3], p3, j4)

        qp(0, 0)()
        kp(0, 0)()

        # unit (p0,c0): kp0 key-blocks just in time (QKT ktp needs
        # kp0(ktp//2)); v-projection at blocks 4-7 (xv arrives late; the
        # AVs consuming it ride in unit (p1,c0)); pair-1 j0 at the end.
        u0 = [[] for _ in range(8)]
        u0[1].append(kp(0, 1))
        u0[3].append(kp(0, 2))
        u0[5].append(kp(0, 3))
        for i in range(16):
            u0[4 + i // 4].append(vproj_block(i))
        u0[6].append(qp(1, 0))
        u0[7].append(kp(1, 0))
        attn_unit(0, 0, u0)

        # unit (p1,c0): rest of kp1 just in time, pair-2 j0 at the end
        u1 = [[] for _ in range(8)]
        u1[1].append(kp(1, 1))
        u1[3].append(kp(1, 2))
        u1[5].append(kp(1, 3))
        u1[6].append(qp(2, 0))
        u1[7].append(kp(2, 0))
        attn_unit(1, 0, u1)

        # unit (p2,c0): rest of kp2, plus q projections for chunk 1
        u2 = [[] for _ in range(8)]
        u2[1].append(kp(2, 1))
        u2[3].append(kp(2, 2))
        u2[5].append(kp(2, 3))
        u2[6].append(qp(0, 1))
        u2[7].append(qp(1, 1))
        attn_unit(2, 0, u2)

        # chunks 1..3. normalize(p2,c) lands in unit (p0,c+1) block 6, so
        # out-proj(c) rides in unit (p1,c+1) blocks 1..6. q projections for
        # later chunks trickle in one unit ahead of their consumer.
        for j4 in range(1, NCH):
            ua = [[] for _ in range(8)]
            attn_unit(0, j4, ua)
            ub = [[] for _ in range(8)]
            ub[0].append(qp(2, j4))
            for mt in range(C // P):
                ub[1 + mt].append(outproj_block(j4 - 1, mt))
            attn_unit(1, j4, ub)
            uc = [[] for _ in range(8)]
            if j4 < NCH - 1:
                uc[6].append(qp(0, j4 + 1))
                uc[7].append(qp(1, j4 + 1))
            attn_unit(2, j4, uc)

        # flush: AV+normalize of the last unit, then final chunk's out-proj
        flush_av()
        for mt in range(C // P):
            outproj_block(NCH - 1, mt)()

        if DEBUG:
            for nm, t in (("dbg_qT", qT_t[0]), ("dbg_xn", xn_t[0])):
                dc = ost.tile([P, NQ], f32, tag="dbgc", name="dbgc", bufs=1)
                nc.vector.tensor_copy(dc, t)
                nc.sync.dma_start(out=dbg_d[nm][:, :], in_=dc)

    nc.finalize()
    return nc


def _get_prog():
    global _prog
    if _prog is None:
        _prog = _build()
    return _prog


def _shard_inputs(query, key, value, Wq, Wk, Wv, Wo):
    from ml_dtypes import bfloat16

    in_maps = []
    for core in range(8):
        b, hg = core // 2, core % 2
        sl = slice(hg * HB, (hg + 1) * HB)
        in_maps.append(
            {
                "xq": np.ascontiguousarray(query[b].T).astype(bfloat16),
                "xk": np.ascontiguousarray(key[b].T).astype(bfloat16),
                "xv": np.ascontiguousarray(value[b].T).astype(bfloat16),
                "wq": np.ascontiguousarray((Wq[sl, :] * SCALE).T).astype(bfloat16),
                "wk": np.ascontiguousarray(Wk[sl, :].T).astype(bfloat16),
                "wv": np.ascontiguousarray(Wv[sl, :].T).astype(bfloat16),
                "wo": np.ascontiguousarray(Wo[:, sl].T).astype(bfloat16),
            }
        )
    return in_maps


def kernel(query, key, value, Wq, Wk, Wv, Wo, bo):
    query, key, value = np.asarray(query), np.asarray(key), np.asarray(value)
    Wq, Wk, Wv, Wo = np.asarray(Wq), np.asarray(Wk), np.asarray(Wv), np.asarray(Wo)
    bo = np.asarray(bo).astype(np.float32)

    from concourse.bass_utils import run_bass_kernel_spmd

    nc = _get_prog()
    in_maps = _shard_inputs(query, key, value, Wq, Wk, Wv, Wo)
    res = run_bass_kernel_spmd(nc, in_maps, list(range(8))).results

    out = np.empty((B, NQ, C), np.float32)
    for b in range(B):
        acc = res[2 * b]["out"].astype(np.float32) + res[2 * b + 1]["out"].astype(
            np.float32
        )
        out[b] = acc.T + bo[None, :]
    return out
